# revision 26
# baseline (speedup 1.0000x reference)
"""Trainium2 Bass kernel for nn_IntensityLoss (bilateral-filter intensity loss).

Math (window sums use raw r_weights; the 1/25 normalizations cancel):
  A  = sum_t w_t                (25-tap sum, per pixel)
  Bf = sum_t fake_t  w_t ; Cf = sum_t fake_t^2  w_t   (taps = 5x5 shifted copies)
  Bg, Cg  likewise for gamma_hdr
  Bh = sum_t H_t w_t  with  H = hdr_original_im ** (1 - f)   (zero-padded)
  Vx  = max(Cx*A - Bx^2, 0) + eps*A^2        (= A^2 * (var + eps))
  num = K * sqrt(Vg) * (Bh + eps*A)          (K = gray_max / f)
  den = A * sqrt(Vf) + num
  r   = num / den                            (= 1 - std_fake/(std_fake+std_obj))
  out = sum(r * (A-1)) / sum(A-1)            (global over B*H*W pixels)

Sharding: core c handles batch b=c//2, rows [256*(c%2), +256).  Each core pads
to 275 "virtual" rows (11 chunks x 25 rows); pad rows get tap weights
{0.25, 24x 1/32} so A=1 exactly -> w_blf=0 -> no contribution.

Layout: "diagonal stack" [125 partitions = 5 row-shifts x 25 rows, 512 cols].
All inputs pre-cast to bf16 on host.  Per chunk, 5 product planes
(w*f, w*f^2, w*g, w*g^2, w*h) are formed as single long [125,5,512] muls,
split across DVE and GpSimd; PE reduces over taps with sparse selector
matmuls accumulating in PSUM; per-group epilogue in fp32 with fused
scalar_tensor_tensor ops and approximate reciprocal.
"""

import sys

sys.path.insert(0, "/opt/trn_rl_repo")

import numpy as np
import ml_dtypes

import concourse.bass as bass
import concourse.bacc as bacc
import concourse.tile as tile
from concourse import mybir
from concourse.bass_utils import run_bass_kernel_spmd

F32 = mybir.dt.float32
BF16 = mybir.dt.bfloat16
AF = mybir.ActivationFunctionType
ALU = mybir.AluOpType
AX = mybir.AxisListType

EPS = 1e-5
EPS_SQRT = float(np.sqrt(np.float32(EPS)))
H_IMG = 512
W_IMG = 512
B_SZ = 4
N_CORES = 8
RPC = 256          # real rows per core
QR = 25            # rows per chunk
NCH = 11           # chunks per core (275 virtual rows)
VROWS = NCH * QR   # 275
PROWS = 280        # padded image rows staged per core
PCOLS = 516        # padded image cols

_CACHE = {}


def _win(t_ap, nparts, per_part, off, nb, width=512):
    """Overlapping-window view of an SBUF tile: [nparts, nb, width] where
    element (p, b, c) = tile[p, off + b + c]."""
    return bass.AP(
        tensor=t_ap.tensor,
        offset=t_ap.offset + off,
        ap=[[per_part, nparts], [1, nb], [1, width]],
    )


def _build_nc():
    nc = bacc.Bacc(None)
    wslab = nc.declare_dram_parameter("wslab", [5, VROWS, 5, W_IMG], BF16, isOutput=False)
    imfg = nc.declare_dram_parameter("imfg", [2, PROWS, PCOLS], BF16, isOutput=False)
    imh = nc.declare_dram_parameter("imh", [PROWS, PCOLS], BF16, isOutput=False)
    hmask = nc.declare_dram_parameter("hmask", [PROWS, 1], F32, isOutput=False)
    gray = nc.declare_dram_parameter("gray", [H_IMG, W_IMG], F32, isOutput=False)
    scal = nc.declare_dram_parameter("scal", [1, 4], F32, isOutput=False)
    stat = nc.declare_dram_parameter("stat", [5, 125, 125], BF16, isOutput=False)
    out = nc.declare_dram_parameter("out", [125, 2], F32, isOutput=True)

    himg = nc.dram_tensor("himg", [PROWS, PCOLS], BF16)

    with tile.TileContext(nc) as tc:
        with (
            tc.tile_pool(name="singles", bufs=1) as singles,
            tc.tile_pool(name="prep", bufs=2) as prep,
            tc.tile_pool(name="chunk", bufs=3) as chunk,
            tc.tile_pool(name="prod", bufs=3) as prod,
            tc.tile_pool(name="stats", bufs=2) as statp,
            tc.tile_pool(name="epi", bufs=2) as epi,
            tc.tile_pool(name="psA", bufs=1, space="PSUM") as psum_stats,
            tc.tile_pool(name="psB", bufs=1, space="PSUM") as psum_misc,
        ):
            # ---------- phase 0: scalars, gray max, H image ----------
            ones = singles.tile([1, 128], F32)
            nc.vector.memset(ones[:], 1.0)

            sc = singles.tile([1, 4], F32)
            nc.sync.dma_start(out=sc[:], in_=scal[:])

            # broadcast 1-f and 1/f to all partitions via PE
            f1m_bc = singles.tile([128, 1], F32)
            finv_bc = singles.tile([128, 1], F32)
            ps_bc = psum_misc.tile([128, 1], F32, tag="bc")
            nc.tensor.matmul(ps_bc[:], ones[:], sc[0:1, 0:1], start=True, stop=True)
            nc.scalar.copy(f1m_bc[:], ps_bc[:])
            ps_bc2 = psum_misc.tile([128, 1], F32, tag="bc", name="ps_bc2")
            nc.tensor.matmul(ps_bc2[:], ones[:], sc[0:1, 1:2], start=True, stop=True)
            nc.scalar.copy(finv_bc[:], ps_bc2[:])

            # gray max over the full batch image (issued on the scalar queue so
            # the sync queue reaches the chunk DMAs sooner)
            gt = prep.tile([128, 2048], F32)
            nc.scalar.dma_start(
                out=gt[:],
                in_=bass.AP(tensor=gray, offset=0, ap=[[2048, 128], [1, 2048]]),
            )
            gm = singles.tile([128, 1], F32)
            nc.vector.tensor_reduce(gm[:], gt[:], axis=AX.X, op=ALU.max)
            gmr = singles.tile([1, 128], F32)
            nc.scalar.dma_start(out=gmr[:], in_=gm[:])
            gms = singles.tile([1, 1], F32)
            nc.vector.tensor_reduce(gms[:], gmr[:], axis=AX.X, op=ALU.max)
            gm_bc = singles.tile([128, 1], F32)
            ps_bc3 = psum_misc.tile([128, 1], F32, tag="bc", name="ps_bc3")
            nc.tensor.matmul(ps_bc3[:], ones[:], gms[0:1, 0:1], start=True, stop=True)
            nc.scalar.copy(gm_bc[:], ps_bc3[:])
            k_sb = singles.tile([128, 1], F32)
            nc.vector.tensor_mul(k_sb[:], gm_bc[:], finv_bc[:])

            # H = (hdr ** (1-f)) with zero padding, stored to DRAM in bf16
            row_tiles = [(0, 128), (128, 128), (256, PROWS - 256)]
            for r0, p in row_tiles:
                ht = prep.tile([128, PCOLS], BF16, tag="ht")
                nc.scalar.dma_start(out=ht[:p, :], in_=imh[r0 : r0 + p, :])
                lt = prep.tile([128, PCOLS], F32, tag="lt")
                nc.scalar.activation(lt[:p, :], ht[:p, :], AF.Ln)
                et = prep.tile([128, PCOLS], BF16, tag="et")
                nc.scalar.activation(et[:p, :], lt[:p, :], AF.Exp, scale=f1m_bc[:p, :])
                hm = prep.tile([128, 1], F32, tag="hm")
                nc.scalar.dma_start(out=hm[:p, :], in_=hmask[r0 : r0 + p, :])
                nc.vector.tensor_scalar_mul(et[:p, :], et[:p, :], hm[:p, 0:1])
                nc.vector.memset(et[:p, 0:2], 0.0)
                nc.vector.memset(et[:p, 514:516], 0.0)
                nc.sync.dma_start(out=himg[r0 : r0 + p, :], in_=et[:p, :])

            # stationary selector matrices
            st_all = singles.tile([125, 5, 125], BF16)
            nc.sync.dma_start(
                out=st_all[:],
                in_=bass.AP(
                    tensor=stat,
                    offset=0,
                    ap=[[125, 125], [125 * 125, 5], [1, 125]],
                ),
            )

            # running reduction accumulators [125, 2]: col0 sum(contrib), col1 sum(A)
            red = singles.tile([125, 2], F32)
            nc.vector.memset(red[:], 0.0)


            STATS = ["A", "Bf", "Cf", "Bg", "Cg", "Bh"]

            # ---------- phase 1: chunks (software pipelined) ----------
            # loads+squares for chunk c+1 are emitted BEFORE products/epilogue
            # of chunk c, so the scalar/sync queues run a chunk ahead and the
            # group-end epilogue never stalls the next chunk's DVE products.
            def load_chunk(c):
                cr0 = c * QR
                wt = chunk.tile([125, 5, 512], BF16, tag="wt", name=f"wt{c}")
                nc.sync.dma_start(
                    out=wt[:],
                    in_=bass.AP(
                        tensor=wslab,
                        offset=cr0 * 5 * W_IMG,
                        ap=[[VROWS * 5 * W_IMG, 5], [5 * W_IMG, QR], [1, 5 * W_IMG]],
                    ),
                )
                rfg = chunk.tile([125, 2, PCOLS], BF16, tag="rfg", name=f"rfg{c}")
                for im in range(2):
                    nc.sync.dma_start(
                        out=rfg[:, im, :],
                        in_=bass.AP(
                            tensor=imfg,
                            offset=im * PROWS * PCOLS + cr0 * PCOLS,
                            ap=[[PCOLS, 5], [PCOLS, QR], [1, PCOLS]],
                        ),
                    )
                rh = chunk.tile([125, PCOLS], BF16, tag="rh", name=f"rh{c}")
                nc.scalar.dma_start(
                    out=rh[:],
                    in_=bass.AP(
                        tensor=himg,
                        offset=cr0 * PCOLS,
                        ap=[[PCOLS, 5], [PCOLS, QR], [1, PCOLS]],
                    ),
                )
                sqf = chunk.tile([125, PCOLS], BF16, tag="sqf", name=f"sqf{c}")
                nc.scalar.activation(sqf[:], rfg[:, 0, :], AF.Square)
                sqg = chunk.tile([125, PCOLS], BF16, tag="sqg", name=f"sqg{c}")
                nc.scalar.activation(sqg[:], rfg[:, 1, :], AF.Square)
                return wt, rfg, rh, sqf, sqg

            def compute_chunk(c, tiles):
                s = c % 5
                g = c // 5
                last_s = 4 if g < 2 else 0
                wt, rfg, rh, sqf, sqg = tiles

                pf1 = prod.tile([125, 5, 512], BF16, tag="pf1", name=f"pf1_{c}")
                pf2 = prod.tile([125, 5, 512], BF16, tag="pf2", name=f"pf2_{c}")
                pg1 = prod.tile([125, 5, 512], BF16, tag="pg1", name=f"pg1_{c}")
                pg2 = prod.tile([125, 5, 512], BF16, tag="pg2", name=f"pg2_{c}")
                ph = prod.tile([125, 5, 512], BF16, tag="ph", name=f"ph_{c}")

                wf = _win(rfg[:], 125, 2 * PCOLS, 0, 5)
                wg = _win(rfg[:], 125, 2 * PCOLS, PCOLS, 5)
                wsf = _win(sqf[:], 125, PCOLS, 0, 5)
                wsg = _win(sqg[:], 125, PCOLS, 0, 5)
                wh3 = _win(rh[:], 125, PCOLS, 0, 3)
                wh2 = _win(rh[:], 125, PCOLS, 3, 2)

                # all products on DVE (gpsimd contends for SBUF ports and
                # slows DVE more than it helps)
                nc.vector.tensor_mul(pf1[:], wf, wt[:])
                nc.vector.tensor_mul(pg1[:], wg, wt[:])
                nc.vector.tensor_mul(pf2[:], wsf, wt[:])
                nc.vector.tensor_mul(pg2[:], wsg, wt[:])
                nc.vector.tensor_mul(ph[:, 0:3, :], wh3, wt[:, 0:3, :])
                nc.vector.tensor_mul(ph[:, 3:5, :], wh2, wt[:, 3:5, :])

                if s == 0:
                    state["ps"] = {
                        name: psum_stats.tile(
                            [125, 512], F32, tag=f"ps{name}", name=f"ps{name}_{g}"
                        )
                        for name in STATS
                    }
                ps = state["ps"]
                movs = {
                    "A": wt,
                    "Bf": pf1,
                    "Bg": pg1,
                    "Cf": pf2,
                    "Cg": pg2,
                    "Bh": ph,   # last: waits on himg early in the run
                }
                first = s == 0
                last = s == last_s
                for name, mov in movs.items():
                    for b in range(5):
                        mm = nc.tensor.matmul(
                            ps[name][:],
                            st_all[:, s, :],
                            mov[:, b, :],
                            start=(first and b == 0),
                            stop=(last and b == 4),
                        )
                        mm.is_weight_onezero = True

                # ---------- per-group epilogue ----------
                if last:
                    nrows = 125 if g < 2 else QR
                    # PSUM->SBUF copies on scalar (gpsimd cannot read PSUM);
                    # the B^2 squares ride the otherwise idle gpsimd engine
                    S = {}
                    for name in STATS:
                        S[name] = statp.tile(
                            [125, 512], F32, tag=f"S{name}", name=f"S{name}_{g}"
                        )
                        nc.scalar.copy(S[name][:], ps[name][:])
                    A = S["A"]

                    e2 = epi.tile([125, 512], F32, tag="e2")
                    nc.scalar.activation(e2[:], A[:], AF.Square, scale=EPS_SQRT)
                    b2f = epi.tile([125, 512], F32, tag="b2f")
                    nc.gpsimd.tensor_mul(b2f[:], S["Bf"][:], S["Bf"][:])
                    b2g = epi.tile([125, 512], F32, tag="b2g")
                    nc.gpsimd.tensor_mul(b2g[:], S["Bg"][:], S["Bg"][:])

                    vf = epi.tile([125, 512], F32, tag="vf")
                    nc.vector.tensor_mul(vf[:], S["Cf"][:], A[:])
                    nc.vector.tensor_tensor(vf[:], vf[:], b2f[:], op=ALU.subtract)
                    nc.vector.scalar_tensor_tensor(
                        vf[:], in0=vf[:], scalar=0.0, in1=e2[:], op0=ALU.max, op1=ALU.add
                    )
                    sf = epi.tile([125, 512], F32, tag="sf")
                    nc.scalar.activation(sf[:], vf[:], AF.Sqrt)

                    vg = epi.tile([125, 512], F32, tag="vg")
                    nc.vector.tensor_mul(vg[:], S["Cg"][:], A[:])
                    nc.vector.tensor_tensor(vg[:], vg[:], b2g[:], op=ALU.subtract)
                    nc.vector.scalar_tensor_tensor(
                        vg[:], in0=vg[:], scalar=0.0, in1=e2[:], op0=ALU.max, op1=ALU.add
                    )
                    sg = epi.tile([125, 512], F32, tag="sg")
                    nc.scalar.activation(sg[:], vg[:], AF.Sqrt)

                    # th = Bh + eps*A
                    th = epi.tile([125, 512], F32, tag="th")
                    nc.vector.scalar_tensor_tensor(
                        th[:], in0=A[:], scalar=EPS, in1=S["Bh"][:],
                        op0=ALU.mult, op1=ALU.add,
                    )
                    # num = (sg * K) * th
                    num = epi.tile([125, 512], F32, tag="num")
                    nc.vector.scalar_tensor_tensor(
                        num[:], in0=sg[:], scalar=k_sb[0:125, 0:1], in1=th[:],
                        op0=ALU.mult, op1=ALU.mult,
                    )
                    # den = A*sf + num ; recip.  (rows >= nrows may divide by 0;
                    # they are excluded from the accumulated sums below)
                    den = epi.tile([125, 512], F32, tag="den")
                    nc.vector.tensor_mul(den[:], A[:], sf[:])
                    nc.vector.tensor_add(den[:], den[:], num[:])
                    nc.vector.reciprocal_approx_fast(
                        den[0:nrows, :], den[0:nrows, :]
                    )
                    nc.vector.tensor_mul(
                        num[0:nrows, :], num[0:nrows, :], den[0:nrows, :]
                    )  # r
                    # contrib = (A-1)*r, with fused row-sum
                    contrib = epi.tile([125, 512], F32, tag="contrib")
                    racc1 = epi.tile([125, 1], F32, tag="racc1")
                    nc.vector.scalar_tensor_tensor(
                        contrib[0:nrows, :], in0=A[0:nrows, :], scalar=-1.0,
                        in1=num[0:nrows, :], op0=ALU.add, op1=ALU.mult,
                        accum_out=racc1[0:nrows, :],
                    )
                    # sum(A) per row on the Scalar engine (host subtracts the
                    # 512-per-row constant to get sum(A-1))
                    scrapA = epi.tile([125, 512], BF16, tag="scrapA")
                    racc2 = epi.tile([125, 1], F32, tag="racc2")
                    nc.scalar.activation(
                        scrapA[0:nrows, :], A[0:nrows, :], AF.Copy,
                        accum_out=racc2[0:nrows, :],
                    )
                    nc.vector.tensor_add(
                        red[0:nrows, 0:1], red[0:nrows, 0:1], racc1[0:nrows, :]
                    )
                    nc.vector.tensor_add(
                        red[0:nrows, 1:2], red[0:nrows, 1:2], racc2[0:nrows, :]
                    )

            state = {}
            pending = load_chunk(0)
            for c in range(NCH):
                nxt = load_chunk(c + 1) if c + 1 < NCH else None
                compute_chunk(c, pending)
                pending = nxt

            nc.sync.dma_start(out=out[:], in_=red[:])

    nc.compile()
    return nc


def _host_inputs(fake, gamma_hdr, hdr_original_im, r_weights, f_factors,
                 hdr_original_gray):
    """Build the 8 per-core input dicts (bf16 pre-cast, layout prep only)."""
    stat_np = np.zeros((5, 125, 125), dtype=np.float32)
    for s in range(5):
        for a in range(5):
            for q in range(25):
                stat_np[s, a * 25 + q, s * 25 + q] = 1.0
    stat_np = stat_np.astype(ml_dtypes.bfloat16)

    def padimg(x, cval):
        return np.pad(x, ((2, 22), (2, 2)), constant_values=cval)

    in_maps = []
    for c in range(N_CORES):
        b = c // 2
        r0 = (c % 2) * RPC
        slab = np.empty((5, 5, VROWS, W_IMG), dtype=np.float32)
        slab[:, :, :RPC, :] = r_weights[b, :, r0 : r0 + RPC, :].reshape(
            5, 5, RPC, W_IMG
        )
        # pad rows: tap (0,0)=0.25, rest 1/32 -> A = 1 exactly in bf16/f32
        slab[:, :, RPC:, :] = 1.0 / 32.0
        slab[0, 0, RPC:, :] = 0.25
        slab = np.ascontiguousarray(slab.transpose(0, 2, 1, 3)).astype(
            ml_dtypes.bfloat16
        )  # [a, row, b, col]

        pf = padimg(fake[b, 0], 0.0)[r0 : r0 + PROWS]
        pg = padimg(gamma_hdr[b, 0], 0.0)[r0 : r0 + PROWS]
        imfg = np.ascontiguousarray(
            np.stack([pf, pg]).astype(ml_dtypes.bfloat16)
        )
        ph = padimg(hdr_original_im[b, 0], 1.0)[r0 : r0 + PROWS].astype(
            ml_dtypes.bfloat16
        )
        gidx = r0 + np.arange(PROWS)
        hm = ((gidx >= 2) & (gidx <= 513)).astype(np.float32).reshape(PROWS, 1)

        f = float(f_factors[b])
        scal = np.array([[1.0 - f, 1.0 / f, 0.0, 0.0]], dtype=np.float32)

        in_maps.append(
            {
                "wslab": np.ascontiguousarray(slab),
                "imfg": imfg,
                "imh": np.ascontiguousarray(ph),
                "hmask": hm,
                "gray": np.ascontiguousarray(hdr_original_gray[b, 0]),
                "scal": scal,
                "stat": stat_np,
            }
        )
    return in_maps


def kernel_run(inputs, **spmd_kwargs):
    """Returns (scalar_result, BassKernelResults)."""
    if "nc" not in _CACHE:
        _CACHE["nc"] = _build_nc()
    nc = _CACHE["nc"]
    in_maps = _host_inputs(**inputs)
    res = run_bass_kernel_spmd(nc, in_maps, list(range(N_CORES)), **spmd_kwargs)
    s1 = 0.0
    s2 = 0.0
    for r in res.results:
        o = np.asarray(r["out"], dtype=np.float64)
        s1 += o[:, 0].sum()
        s2 += o[:, 1].sum() - 512.0 * VROWS   # sum(A) -> sum(A-1)
    return np.float32(s1 / s2), res


def kernel(**inputs):
    result, _ = kernel_run(inputs)
    return result
